# revision 1
# baseline (speedup 1.0000x reference)
"""Trainium2 Bass kernel for complex depthwise batchnorm (training-mode stats).

Data-parallel over batch N across 8 NeuronCores. Per core:
  phase A: stream the [2048, 2056] shard, accumulate per-column
           {sum xr, sum xi, sum xr^2, sum xi^2, sum xr*xi} via ones-vector
           matmuls into PSUM (fp32 matmuls for plain sums, bf16 for the
           three product sums — squares/cross are written bf16 by ACT/DVE).
  AllReduce (41KB) of the 5x2056 sums across cores.
  coefficient math on [8, 257]: 2x2 inverse-sqrt covariance whitening +
           affine mixing collapsed to y = Z@x + b' per column.
  phase B: stream the shard again, yr = Zrr*xr + Zri*xi + br',
           yi = Zir*xr + Zii*xi + bi' with coefficients broadcast across
           partitions via PE ones-broadcast; ops split across DVE + GpSimd.
"""

import numpy as np

N, C, F = 16384, 8, 257
D = C * F  # 2056
N_CORES = 8
NS = N // N_CORES  # 2048
P = 128
T = NS // P  # 16 tiles per core
EPS = 1e-6
DELTA_MAX = 1e8

# free-dim chunks for reduction matmuls (PSUM bank = 512 fp32)
# 4 full 512-wide chunks cover [0, 2048); the 8-col tail is packed separately
RED_CHUNKS = [(c * 512, 512) for c in range(4)]
TAIL_OFF, TAIL_W = 2048, D - 2048  # 8 columns
# column halves for phase B elementwise work
HALVES = [(0, D // 2), (D // 2, D - D // 2)]

_CACHE = {}


def _build():
    import concourse.bacc as bacc
    import concourse.tile as tile
    import concourse.mybir as mybir

    f32 = mybir.dt.float32
    bf16 = mybir.dt.bfloat16
    Alu = mybir.AluOpType
    Act = mybir.ActivationFunctionType

    nc = bacc.Bacc("TRN2", target_bir_lowering=False, debug=False,
                   num_devices=N_CORES)

    xr = nc.dram_tensor("xr", [NS, D], f32, kind="ExternalInput").ap()
    xi = nc.dram_tensor("xi", [NS, D], f32, kind="ExternalInput").ap()
    wrr = nc.dram_tensor("wrr", [C, F], f32, kind="ExternalInput").ap()
    wri = nc.dram_tensor("wri", [C, F], f32, kind="ExternalInput").ap()
    wii = nc.dram_tensor("wii", [C, F], f32, kind="ExternalInput").ap()
    br = nc.dram_tensor("br", [C, F], f32, kind="ExternalInput").ap()
    bi = nc.dram_tensor("bi", [C, F], f32, kind="ExternalInput").ap()
    yr = nc.dram_tensor("yr", [NS, D], f32, kind="ExternalOutput").ap()
    yi = nc.dram_tensor("yi", [NS, D], f32, kind="ExternalOutput").ap()

    with tile.TileContext(nc) as tc:
        with (
            tc.tile_pool(name="const", bufs=1) as cpool,
            tc.tile_pool(name="inp", bufs=3) as inp,
            tc.tile_pool(name="sq", bufs=8) as sqp,
            tc.tile_pool(name="tmpb", bufs=2) as tmpb,
            tc.tile_pool(name="small", bufs=1) as smp,
            tc.tile_pool(name="ctmp", bufs=6) as ctp,
            tc.tile_pool(name="dram", bufs=1, space="DRAM") as dram,
        ):
            ones_f = cpool.tile([P, 1], f32, name="ones_f")
            nc.vector.memset(ones_f[:], 1.0)
            ones_b = cpool.tile([P, 1], bf16, name="ones_b")
            nc.vector.memset(ones_b[:], 1.0)
            ones_row = cpool.tile([1, P], f32, name="ones_row")
            nc.vector.memset(ones_row[:], 1.0)

            # ---------------- phase A: local sums ----------------
            # PE matmul outputs must start at partition 0/32/64. Quantities:
            #   accA: q0=sum(xr)@p0, q1=sum(xi)@p32, q2=sum(xr^2)@p64
            #   accB: q3=sum(xi^2)@p0, q4=sum(xr*xi)@p32,
            #         all 5 tails (cols 2048:2056) @p64, free offset q*16
            cc_in = dram.tile([5, D], f32, name="cc_in")
            cc_out = dram.tile([5, D], f32, name="cc_out", addr_space="Shared")
            with tc.tile_pool(name="acc", bufs=1, space="PSUM") as accp:
                accA = accp.tile([65, 2048], f32, name="accA")  # 4 banks
                accB = accp.tile([65, 2048], f32, name="accB")  # 4 banks
                # (tile, base partition, tail free offset) per quantity
                QSLOT = [(accA, 0), (accA, 32), (accA, 64),
                         (accB, 0), (accB, 32)]

                for i in range(T):
                    xr_t = inp.tile([P, D], f32, tag="xr", name=f"xr_{i}")
                    nc.sync.dma_start(out=xr_t[:], in_=xr[i * P:(i + 1) * P, :])
                    xi_t = inp.tile([P, D], f32, tag="xi", name=f"xi_{i}")
                    nc.sync.dma_start(out=xi_t[:], in_=xi[i * P:(i + 1) * P, :])

                    st, fin = (i == 0), (i == T - 1)

                    def red(q, rhs_ap, ones_t):
                        tile_, p = QSLOT[q]
                        off, w = CUR_CHUNK
                        if off < TAIL_OFF:
                            nc.tensor.matmul(tile_[p:p + 1, off:off + w],
                                             lhsT=ones_t[:], rhs=rhs_ap,
                                             start=st, stop=fin)
                        else:
                            # all 5 tails share one 2KB zero region at
                            # accB partition 64: q0's first matmul zeroes it,
                            # q4's last matmul closes the group
                            nc.tensor.matmul(accB[64:65, q * 8:q * 8 + w],
                                             lhsT=ones_t[:], rhs=rhs_ap,
                                             start=(st and q == 0),
                                             stop=(fin and q == 4))

                    for off, w in RED_CHUNKS + [(TAIL_OFF, TAIL_W)]:
                        CUR_CHUNK = (off, w)
                        sl = slice(off, off + w)
                        red(0, xr_t[:, sl], ones_f)
                        red(1, xi_t[:, sl], ones_f)
                        sqr = sqp.tile([P, 512], bf16, tag="sqr",
                                       name=f"sqr_{i}_{off}")
                        nc.scalar.activation(sqr[:, 0:w], xr_t[:, sl],
                                             Act.Square)
                        red(2, sqr[:, 0:w], ones_b)
                        sqi = sqp.tile([P, 512], bf16, tag="sqi",
                                       name=f"sqi_{i}_{off}")
                        nc.scalar.activation(sqi[:, 0:w], xi_t[:, sl],
                                             Act.Square)
                        red(3, sqi[:, 0:w], ones_b)
                        crs = sqp.tile([P, 512], bf16, tag="crs",
                                       name=f"crs_{i}_{off}")
                        nc.vector.tensor_tensor(crs[:, 0:w], xr_t[:, sl],
                                                xi_t[:, sl], Alu.mult)
                        red(4, crs[:, 0:w], ones_b)

                # partition-aligned PSUM -> SBUF copies, then row-gather DMAs
                # (staged in the phase-B temp slots, idle at this point)
                sums_a = tmpb.tile([65, 2048], f32, tag="t1", name="sums_a")
                sums_b = tmpb.tile([65, 2048], f32, tag="t2", name="sums_b")
                nc.vector.tensor_copy(sums_a[0:1, :], accA[0:1, :])
                nc.scalar.copy(sums_a[32:33, :], accA[32:33, :])
                nc.vector.tensor_copy(sums_a[64:65, :], accA[64:65, :])
                nc.scalar.copy(sums_b[0:1, :], accB[0:1, :])
                nc.vector.tensor_copy(sums_b[32:33, :], accB[32:33, :])
                nc.scalar.copy(sums_b[64:65, 0:40], accB[64:65, 0:40])

            SB_SLOT = [(sums_a, 0), (sums_a, 32), (sums_a, 64),
                       (sums_b, 0), (sums_b, 32)]
            for q, (tile_, p) in enumerate(SB_SLOT):
                nc.sync.dma_start(out=cc_in[q:q + 1, 0:TAIL_OFF],
                                  in_=tile_[p:p + 1, :])
                nc.sync.dma_start(
                    out=cc_in[q:q + 1, TAIL_OFF:D],
                    in_=sums_b[64:65, q * 8:q * 8 + TAIL_W])

            # ---------------- all-reduce ----------------
            nc.gpsimd.collective_compute(
                "AllReduce",
                Alu.add,
                replica_groups=[list(range(N_CORES))],
                ins=[cc_in[:].opt()],
                outs=[cc_out[:].opt()],
            )
            cc_cf = cc_out[:].rearrange("q (c f) -> (q c) f", c=C)

            def load_cf(name, src):
                t = smp.tile([C, F], f32, name=name)
                nc.sync.dma_start(out=t[:], in_=src)
                return t

            s_xr = load_cf("s_xr", cc_cf[0 * C:1 * C, :])
            s_xi = load_cf("s_xi", cc_cf[1 * C:2 * C, :])
            s_rr = load_cf("s_rr", cc_cf[2 * C:3 * C, :])
            s_ii = load_cf("s_ii", cc_cf[3 * C:4 * C, :])
            s_ri = load_cf("s_ri", cc_cf[4 * C:5 * C, :])
            w_rr = load_cf("w_rr", wrr[:, :])
            w_ri = load_cf("w_ri", wri[:, :])
            w_ii = load_cf("w_ii", wii[:, :])
            b_r = load_cf("b_r", br[:, :])
            b_i = load_cf("b_i", bi[:, :])

            # ---------------- coefficient math on [C, F] ----------------
            inv_n = 1.0 / N
            V = nc.vector
            S = nc.scalar

            def keep(name):
                return smp.tile([C, F], f32, name=name)

            def scratch(name):
                return ctp.tile([C, F], f32, tag="ct", name=name)

            mr = keep("mr")
            V.tensor_scalar_mul(mr[:], s_xr[:], inv_n)
            mi = keep("mi")
            V.tensor_scalar_mul(mi[:], s_xi[:], inv_n)

            mr2 = scratch("mr2")
            V.tensor_tensor(mr2[:], mr[:], mr[:], Alu.mult)
            mi2 = scratch("mi2")
            V.tensor_tensor(mi2[:], mi[:], mi[:], Alu.mult)
            mri = scratch("mri")
            V.tensor_tensor(mri[:], mr[:], mi[:], Alu.mult)

            vrr = keep("vrr")
            V.scalar_tensor_tensor(vrr[:], s_rr[:], inv_n, mr2[:],
                                   Alu.mult, Alu.subtract)
            vii = keep("vii")
            V.scalar_tensor_tensor(vii[:], s_ii[:], inv_n, mi2[:],
                                   Alu.mult, Alu.subtract)
            vri = keep("vri")
            V.scalar_tensor_tensor(vri[:], s_ri[:], inv_n, mri[:],
                                   Alu.mult, Alu.subtract)

            tau = keep("tau")
            V.tensor_tensor(tau[:], vrr[:], vii[:], Alu.add)
            d1 = scratch("d1")
            V.tensor_tensor(d1[:], vrr[:], vii[:], Alu.mult)
            vri2 = scratch("vri2")
            V.tensor_tensor(vri2[:], vri[:], vri[:], Alu.mult)
            delta = keep("delta")
            V.tensor_tensor(delta[:], d1[:], vri2[:], Alu.subtract)
            V.tensor_scalar_max(delta[:], delta[:], EPS)
            V.tensor_scalar_min(delta[:], delta[:], DELTA_MAX)

            s_t = keep("s_t")
            S.activation(s_t[:], delta[:], Act.Sqrt)
            targ = scratch("targ")
            V.scalar_tensor_tensor(targ[:], s_t[:], 2.0, tau[:],
                                   Alu.mult, Alu.add)
            t_t = keep("t_t")
            S.activation(t_t[:], targ[:], Act.Sqrt)
            st_t = scratch("st_t")
            V.tensor_tensor(st_t[:], s_t[:], t_t[:], Alu.mult)
            rst = keep("rst")
            V.reciprocal(rst[:], st_t[:])

            a1 = scratch("a1")
            V.tensor_tensor(a1[:], s_t[:], vii[:], Alu.add)
            urr = keep("urr")
            V.tensor_tensor(urr[:], a1[:], rst[:], Alu.mult)
            a2 = scratch("a2")
            V.tensor_tensor(a2[:], s_t[:], vrr[:], Alu.add)
            uii = keep("uii")
            V.tensor_tensor(uii[:], a2[:], rst[:], Alu.mult)
            uri = keep("uri")
            V.scalar_tensor_tensor(uri[:], vri[:], -1.0, rst[:],
                                   Alu.mult, Alu.mult)

            def mix(name, wa, ua, wb, ub):
                g1 = scratch(name + "_g1")
                V.tensor_tensor(g1[:], wa[:], ua[:], Alu.mult)
                g2 = scratch(name + "_g2")
                V.tensor_tensor(g2[:], wb[:], ub[:], Alu.mult)
                z = keep(name)
                V.tensor_tensor(z[:], g1[:], g2[:], Alu.add)
                return z

            zrr = mix("zrr", w_rr, urr, w_ri, uri)
            zri = mix("zri", w_rr, uri, w_ri, uii)
            zir = mix("zir", w_ri, urr, w_ii, uri)
            zii = mix("zii", w_ri, uri, w_ii, uii)

            def bias(name, b0, za, zb):
                h1 = scratch(name + "_h1")
                V.tensor_tensor(h1[:], za[:], mr[:], Alu.mult)
                h2 = scratch(name + "_h2")
                V.tensor_tensor(h2[:], zb[:], mi[:], Alu.mult)
                h3 = scratch(name + "_h3")
                V.tensor_tensor(h3[:], h1[:], h2[:], Alu.add)
                bb = keep(name)
                V.tensor_tensor(bb[:], b0[:], h3[:], Alu.subtract)
                return bb

            brp = bias("brp", b_r, zrr, zri)
            bip = bias("bip", b_i, zir, zii)

            # ---------------- broadcast coeffs to [128, D] ----------------
            # repack each [C, F] coeff into a [1, D] partition-0 row (DMA),
            # then PE ones-broadcast (matmul rhs must sit at partition 0)
            bcs = []
            with tc.tile_pool(name="bps", bufs=4, space="PSUM") as bps:
                for k, coef in enumerate([zrr, zri, zir, zii, brp, bip]):
                    row = smp.tile([1, D], f32, tag="row", name=f"row{k}")
                    nc.sync.dma_start(out=row[0:1, :], in_=coef[:])
                    bc = cpool.tile([P, D], f32, name=f"bc{k}")
                    for off, w in RED_CHUNKS + [(TAIL_OFF, TAIL_W)]:
                        pb = bps.tile([P, 512], f32, tag="pb",
                                      name=f"pb{k}_{off}")
                        nc.tensor.matmul(pb[:, 0:w], lhsT=ones_row[:],
                                         rhs=row[0:1, off:off + w],
                                         start=True, stop=True)
                        nc.scalar.copy(bc[:, off:off + w], pb[:, 0:w])
                    bcs.append(bc)
            bzrr, bzri, bzir, bzii, bbrp, bbip = bcs

            # ---------------- phase B: apply ----------------
            for i in range(T):
                xr_t = inp.tile([P, D], f32, tag="xr", name=f"xr2_{i}")
                nc.sync.dma_start(out=xr_t[:], in_=xr[i * P:(i + 1) * P, :])
                xi_t = inp.tile([P, D], f32, tag="xi", name=f"xi2_{i}")
                nc.sync.dma_start(out=xi_t[:], in_=xi[i * P:(i + 1) * P, :])

                t1 = tmpb.tile([P, D], f32, tag="t1", name=f"t1_{i}")
                t2 = tmpb.tile([P, D], f32, tag="t2", name=f"t2_{i}")

                # full-width ops; xr_t/xi_t overwritten in place after reads.
                # 6 ops on DVE, 2 on GpSimd (POOL is ~2.4x slower per element
                # and shares SBUF ports with DVE).
                nc.vector.tensor_tensor(t1[:], xr_t[:], bzrr[:], Alu.mult)
                nc.gpsimd.tensor_tensor(t2[:], xi_t[:], bzri[:], Alu.mult)
                nc.gpsimd.tensor_tensor(xr_t[:], xr_t[:], bzir[:], Alu.mult)
                nc.vector.tensor_tensor(xi_t[:], xi_t[:], bzii[:], Alu.mult)
                # yr = t1 + t2 + brp
                nc.vector.tensor_tensor(t1[:], t1[:], t2[:], Alu.add)
                nc.vector.tensor_tensor(t1[:], t1[:], bbrp[:], Alu.add)
                # yi = xr_t + xi_t + bip
                nc.vector.tensor_tensor(xr_t[:], xr_t[:], xi_t[:], Alu.add)
                nc.vector.tensor_tensor(xr_t[:], xr_t[:], bbip[:], Alu.add)
                nc.sync.dma_start(out=yr[i * P:(i + 1) * P, :], in_=t1[:])
                nc.sync.dma_start(out=yi[i * P:(i + 1) * P, :], in_=xr_t[:])

    nc.compile()
    return nc


def get_nc():
    if "nc" not in _CACHE:
        _CACHE["nc"] = _build()
    return _CACHE["nc"]


def kernel(xr, xi, Wrr, Wri, Wii, Br, Bi):
    from concourse import bass_utils

    nc = get_nc()
    xr2 = np.ascontiguousarray(np.asarray(xr), dtype=np.float32).reshape(N, D)
    xi2 = np.ascontiguousarray(np.asarray(xi), dtype=np.float32).reshape(N, D)
    params = {
        "wrr": np.ascontiguousarray(np.asarray(Wrr), dtype=np.float32),
        "wri": np.ascontiguousarray(np.asarray(Wri), dtype=np.float32),
        "wii": np.ascontiguousarray(np.asarray(Wii), dtype=np.float32),
        "br": np.ascontiguousarray(np.asarray(Br), dtype=np.float32),
        "bi": np.ascontiguousarray(np.asarray(Bi), dtype=np.float32),
    }
    in_maps = []
    for r in range(N_CORES):
        m = {"xr": xr2[r * NS:(r + 1) * NS], "xi": xi2[r * NS:(r + 1) * NS]}
        m.update(params)
        in_maps.append(m)

    res = bass_utils.run_bass_kernel_spmd(nc, in_maps,
                                          core_ids=list(range(N_CORES)))
    yr_ = np.concatenate([res.results[r]["yr"] for r in range(N_CORES)], axis=0)
    yi_ = np.concatenate([res.results[r]["yi"] for r in range(N_CORES)], axis=0)
    return yr_.reshape(N, C, F), yi_.reshape(N, C, F)



# revision 14
# speedup vs baseline: 1.2422x; 1.2422x over previous
"""Trainium2 Bass kernel for complex depthwise batchnorm (training-mode stats).

v2: single-pass HBM streaming with bf16 SBUF residency.

Data-parallel over batch N across 8 NeuronCores. Per core:
  phase A: stream the [2048, 2056] fp32 shard once (half-tiles of
           [128, 1024|1032]); downconvert to bf16 resident tiles;
           accumulate per-column {sum xr, sum xi, sum xr^2, sum xi^2,
           sum xr*xi} via bf16 ones-matmuls into PSUM (fp32 accum).
  AllReduce (41KB fp32) of the 5x2056 sums across cores.
  coefficient math on [8, 257]: 2x2 inverse-sqrt covariance whitening +
           affine mixing collapsed to y = Z@x + b' per column; coeffs
           broadcast across partitions via PE ones-broadcast (bf16).
  phase B: read resident bf16 tiles, yr = Zrr*xr + Zri*xi + br',
           yi = Zir*xr + Zii*xi + bi' (6 DVE + 2 GpSimd bf16 ops per
           tile), write bf16 outputs; host upcasts to fp32.
"""

import numpy as np

N, C, F = 16384, 8, 257
D = C * F  # 2056
N_CORES = 8
NS = N // N_CORES  # 2048
P = 128
T = NS // P  # 16 row blocks per core
EPS = 1e-6
DELTA_MAX = 1e8

# column halves streamed separately to bound fp32 staging SBUF;
# h1 carries the 8-col tail (2056 = 1024 + 1032)
H1W = D - 1024  # 1032
HALF = [(0, 1024), (1024, H1W)]
# matmul chunk width (PSUM bank = 512 fp32)
CHUNKS = [(0, 512), (512, 512), (1024, 512), (1536, 512)]  # global offsets
TAIL_OFF, TAIL_W = 2048, D - 2048  # 8 columns

_CACHE = {}


def _build():
    import concourse.bacc as bacc
    import concourse.tile as tile
    import concourse.mybir as mybir

    f32 = mybir.dt.float32
    bf16 = mybir.dt.bfloat16
    Alu = mybir.AluOpType
    Act = mybir.ActivationFunctionType

    nc = bacc.Bacc("TRN2", target_bir_lowering=False, debug=False,
                   num_devices=N_CORES)

    xr = nc.dram_tensor("xr", [NS, D], f32, kind="ExternalInput").ap()
    xi = nc.dram_tensor("xi", [NS, D], f32, kind="ExternalInput").ap()
    wrr = nc.dram_tensor("wrr", [C, F], f32, kind="ExternalInput").ap()
    wri = nc.dram_tensor("wri", [C, F], f32, kind="ExternalInput").ap()
    wii = nc.dram_tensor("wii", [C, F], f32, kind="ExternalInput").ap()
    br = nc.dram_tensor("br", [C, F], f32, kind="ExternalInput").ap()
    bi = nc.dram_tensor("bi", [C, F], f32, kind="ExternalInput").ap()
    yr = nc.dram_tensor("yr", [NS, D], bf16, kind="ExternalOutput").ap()
    yi = nc.dram_tensor("yi", [NS, D], bf16, kind="ExternalOutput").ap()

    with tile.TileContext(nc) as tc:
        with (
            tc.tile_pool(name="const", bufs=1) as cpool,
            tc.tile_pool(name="res", bufs=1) as rpool,
            tc.tile_pool(name="inp", bufs=2) as inp,
            tc.tile_pool(name="small", bufs=1) as smp,
            tc.tile_pool(name="ctmp", bufs=4) as ctp,
            tc.tile_pool(name="dram", bufs=1, space="DRAM") as dram,
        ):
            ones_b = cpool.tile([P, 1], bf16, name="ones_b")
            nc.vector.memset(ones_b[:], 1.0)
            ones_row_b = cpool.tile([1, P], bf16, name="ones_row_b")
            nc.vector.memset(ones_row_b[:], 1.0)

            # early (tiny) param loads — before the big streaming DMAs
            def load_cf(name, src, pool=None):
                t = (pool or smp).tile([C, F], f32, name=name)
                nc.sync.dma_start(out=t[:], in_=src)
                return t

            w_rr = load_cf("w_rr", wrr[:, :])
            w_ri = load_cf("w_ri", wri[:, :])
            w_ii = load_cf("w_ii", wii[:, :])
            b_r = load_cf("b_r", br[:, :])
            b_i = load_cf("b_i", bi[:, :])

            # bf16 resident copy of the whole shard (16.4KB/partition x2)
            res_r = [rpool.tile([P, D], bf16, name=f"rr{t}") for t in range(T)]
            res_i = [rpool.tile([P, D], bf16, name=f"ri{t}") for t in range(T)]

            cc_in = dram.tile([5, D], f32, name="cc_in")
            cc_out = dram.tile([5, D], f32, name="cc_out", addr_space="Shared")

            # ---------------- phase A: stream + local sums ----------------
            # PE matmul outputs start at partition 0/32/64:
            #   accA: q0=sum(xr)@p0, q1=sum(xi)@p32, q2=sum(xr^2)@p64
            #   accB: q3=sum(xi^2)@p0, q4=sum(xr*xi)@p32,
            #         all 5 tails (cols 2048:2056) @p64, free offset q*8
            with tc.tile_pool(name="acc", bufs=1, space="PSUM") as accp:
                accA = accp.tile([65, 2048], f32, name="accA")  # 4 banks
                accB = accp.tile([65, 2048], f32, name="accB")  # 4 banks
                QSLOT = [(accA, 0), (accA, 32), (accA, 64),
                         (accB, 0), (accB, 32)]

                sqp = tc.alloc_tile_pool(name="sq", bufs=2)
                for r in range(T):
                    rows = slice(r * P, (r + 1) * P)
                    for h, (goff, w) in enumerate(HALF):
                        gsl = slice(goff, goff + w)
                        xr_t = inp.tile([P, H1W], f32, tag="i1",
                                        name=f"xr_{r}_{h}")
                        nc.sync.dma_start(out=xr_t[:, 0:w], in_=xr[rows, gsl])
                        xi_t = inp.tile([P, H1W], f32, tag="i2",
                                        name=f"xi_{r}_{h}")
                        nc.sync.dma_start(out=xi_t[:, 0:w], in_=xi[rows, gsl])

                        rr = res_r[r][:, gsl]
                        ri = res_i[r][:, gsl]
                        # fp32 -> bf16 residency (ACT + DVE)
                        nc.scalar.copy(rr, xr_t[:, 0:w])
                        nc.vector.tensor_copy(ri, xi_t[:, 0:w])
                        # squares / cross, bf16 (ACT from fp32; DVE/GpSimd bf16)
                        sq_r = sqp.tile([P, H1W], bf16, tag="s1",
                                        name=f"sqr{r}{h}")
                        nc.scalar.activation(sq_r[:, 0:w], xr_t[:, 0:w],
                                             Act.Square)
                        sq_i = sqp.tile([P, H1W], bf16, tag="s2",
                                        name=f"sqi{r}{h}")
                        nc.vector.tensor_tensor(sq_i[:, 0:w], ri, ri, Alu.mult)
                        crs = sqp.tile([P, H1W], bf16, tag="s3",
                                       name=f"crs{r}{h}")
                        nc.gpsimd.tensor_tensor(crs[:, 0:w], rr, ri, Alu.mult)

                        st, fin = (r == 0), (r == T - 1)
                        # rhs source per quantity, sliced locally
                        def qsrc(q, lo, lw):
                            if q == 0:
                                return res_r[r][:, goff + lo:goff + lo + lw]
                            if q == 1:
                                return res_i[r][:, goff + lo:goff + lo + lw]
                            t_ = (sq_r, sq_i, crs)[q - 2]
                            return t_[:, lo:lo + lw]

                        for off, cw in CHUNKS:
                            if not (goff <= off < goff + w):
                                continue
                            lo = off - goff
                            for q, (tile_, p) in enumerate(QSLOT):
                                nc.tensor.matmul(
                                    tile_[p:p + 1, off:off + cw],
                                    lhsT=ones_b[:], rhs=qsrc(q, lo, cw),
                                    start=st, stop=fin)
                        if h == 1:
                            # 8-col tail: all 5 tails share one region at
                            # accB partition 64 (q0 opens, q4 closes)
                            lo = TAIL_OFF - goff
                            for q in range(5):
                                nc.tensor.matmul(
                                    accB[64:65, q * 8:q * 8 + TAIL_W],
                                    lhsT=ones_b[:], rhs=qsrc(q, lo, TAIL_W),
                                    start=(st and q == 0),
                                    stop=(fin and q == 4))

                # dense PSUM -> SBUF copies (one per acc tile; the
                # unwritten partitions between the 0/32/64 rows carry
                # garbage that is never DMA'd), then row DMAs to DRAM
                sqp.release()
                with tc.tile_pool(name="stg", bufs=1) as stg:
                    sA = stg.tile([65, 2048], f32, name="sA")
                    sB = stg.tile([65, 2048], f32, name="sB")
                    nc.vector.tensor_copy(sA[:], accA[:])
                    nc.scalar.copy(sB[:], accB[:])
                    SSLOT = [(sA, 0), (sA, 32), (sA, 64), (sB, 0), (sB, 32)]
                    for q, (tile_, p) in enumerate(SSLOT):
                        nc.sync.dma_start(out=cc_in[q:q + 1, 0:TAIL_OFF],
                                          in_=tile_[p:p + 1, :])
                        nc.sync.dma_start(
                            out=cc_in[q:q + 1, TAIL_OFF:D],
                            in_=sB[64:65, q * 8:q * 8 + TAIL_W])

            # ---------------- all-reduce ----------------
            nc.gpsimd.collective_compute(
                "AllReduce",
                Alu.add,
                replica_groups=[list(range(N_CORES))],
                ins=[cc_in[:].opt()],
                outs=[cc_out[:].opt()],
            )
            cc_cf = cc_out[:].rearrange("q (c f) -> (q c) f", c=C)

            # coefficient-math tiles live in their own pool, opened after
            # phase A's sq/stg pools released their SBUF
            cfp = tc.alloc_tile_pool(name="coef", bufs=1)

            s_xr = load_cf("s_xr", cc_cf[0 * C:1 * C, :], cfp)
            s_xi = load_cf("s_xi", cc_cf[1 * C:2 * C, :], cfp)
            s_rr = load_cf("s_rr", cc_cf[2 * C:3 * C, :], cfp)
            s_ii = load_cf("s_ii", cc_cf[3 * C:4 * C, :], cfp)
            s_ri = load_cf("s_ri", cc_cf[4 * C:5 * C, :], cfp)

            # ---------------- coefficient math on [C, F] ----------------
            inv_n = 1.0 / N
            V = nc.vector
            S = nc.scalar

            def keep(name):
                return cfp.tile([C, F], f32, name=name)

            def scratch(name):
                return ctp.tile([C, F], f32, tag="ct", name=name)

            mr = keep("mr")
            V.tensor_scalar_mul(mr[:], s_xr[:], inv_n)
            mi = keep("mi")
            V.tensor_scalar_mul(mi[:], s_xi[:], inv_n)

            mr2 = scratch("mr2")
            V.tensor_tensor(mr2[:], mr[:], mr[:], Alu.mult)
            mi2 = scratch("mi2")
            V.tensor_tensor(mi2[:], mi[:], mi[:], Alu.mult)
            mri = scratch("mri")
            V.tensor_tensor(mri[:], mr[:], mi[:], Alu.mult)

            vrr = keep("vrr")
            V.scalar_tensor_tensor(vrr[:], s_rr[:], inv_n, mr2[:],
                                   Alu.mult, Alu.subtract)
            vii = keep("vii")
            V.scalar_tensor_tensor(vii[:], s_ii[:], inv_n, mi2[:],
                                   Alu.mult, Alu.subtract)
            vri = keep("vri")
            V.scalar_tensor_tensor(vri[:], s_ri[:], inv_n, mri[:],
                                   Alu.mult, Alu.subtract)

            tau = keep("tau")
            V.tensor_tensor(tau[:], vrr[:], vii[:], Alu.add)
            d1 = scratch("d1")
            V.tensor_tensor(d1[:], vrr[:], vii[:], Alu.mult)
            vri2 = scratch("vri2")
            V.tensor_tensor(vri2[:], vri[:], vri[:], Alu.mult)
            delta = keep("delta")
            V.tensor_tensor(delta[:], d1[:], vri2[:], Alu.subtract)
            V.tensor_scalar_max(delta[:], delta[:], EPS)
            V.tensor_scalar_min(delta[:], delta[:], DELTA_MAX)

            s_t = keep("s_t")
            S.activation(s_t[:], delta[:], Act.Sqrt)
            targ = scratch("targ")
            V.scalar_tensor_tensor(targ[:], s_t[:], 2.0, tau[:],
                                   Alu.mult, Alu.add)
            t_t = keep("t_t")
            S.activation(t_t[:], targ[:], Act.Sqrt)
            st_t = scratch("st_t")
            V.tensor_tensor(st_t[:], s_t[:], t_t[:], Alu.mult)
            rst = keep("rst")
            V.reciprocal(rst[:], st_t[:])

            a1 = scratch("a1")
            V.tensor_tensor(a1[:], s_t[:], vii[:], Alu.add)
            urr = keep("urr")
            V.tensor_tensor(urr[:], a1[:], rst[:], Alu.mult)
            a2 = scratch("a2")
            V.tensor_tensor(a2[:], s_t[:], vrr[:], Alu.add)
            uii = keep("uii")
            V.tensor_tensor(uii[:], a2[:], rst[:], Alu.mult)
            uri = keep("uri")
            V.scalar_tensor_tensor(uri[:], vri[:], -1.0, rst[:],
                                   Alu.mult, Alu.mult)

            def mix(name, wa, ua, wb, ub):
                g1 = scratch(name + "_g1")
                V.tensor_tensor(g1[:], wa[:], ua[:], Alu.mult)
                g2 = scratch(name + "_g2")
                V.tensor_tensor(g2[:], wb[:], ub[:], Alu.mult)
                z = keep(name)
                V.tensor_tensor(z[:], g1[:], g2[:], Alu.add)
                return z

            zrr = mix("zrr", w_rr, urr, w_ri, uri)
            zri = mix("zri", w_rr, uri, w_ri, uii)
            zir = mix("zir", w_ri, urr, w_ii, uri)
            zii = mix("zii", w_ri, uri, w_ii, uii)

            def bias(name, b0, za, zb):
                h1 = scratch(name + "_h1")
                V.tensor_tensor(h1[:], za[:], mr[:], Alu.mult)
                h2 = scratch(name + "_h2")
                V.tensor_tensor(h2[:], zb[:], mi[:], Alu.mult)
                h3 = scratch(name + "_h3")
                V.tensor_tensor(h3[:], h1[:], h2[:], Alu.add)
                bb = keep(name)
                V.tensor_tensor(bb[:], b0[:], h3[:], Alu.subtract)
                return bb

            brp = bias("brp", b_r, zrr, zri)
            bip = bias("bip", b_i, zir, zii)

            # ---------------- broadcast coeffs to [128, D] bf16 ----------
            # [C,F] f32 -> [C,F] bf16 -> [1,D] row (DMA gather) ->
            # PE ones-broadcast -> PSUM -> bf16 bc tile (copy engine rotates)
            bcs = []
            cp_engs = [lambda o, i: nc.scalar.copy(o, i),
                       lambda o, i: nc.vector.tensor_copy(o, i)]
            with tc.tile_pool(name="bps", bufs=4, space="PSUM") as bps:
                ci = 0
                for k, coef in enumerate([zrr, zri, zir, zii, brp, bip]):
                    cb = cfp.tile([C, F], bf16, name=f"cb{k}")
                    (V.tensor_copy if k % 2 else S.copy)(cb[:], coef[:])
                    row = inp.tile([1, D], bf16, tag="i1", name=f"row{k}")
                    nc.sync.dma_start(out=row[0:1, 0:D], in_=cb[:])
                    bc = cpool.tile([P, D], bf16, name=f"bc{k}")
                    for off, cw in CHUNKS + [(TAIL_OFF, TAIL_W)]:
                        pb = bps.tile([P, 512], f32, tag="pb",
                                      name=f"pb{k}_{off}")
                        nc.tensor.matmul(pb[:, 0:cw], lhsT=ones_row_b[:],
                                         rhs=row[0:1, off:off + cw],
                                         start=True, stop=True)
                        cp_engs[ci % 2](bc[:, off:off + cw], pb[:, 0:cw])
                        ci += 1
                    bcs.append(bc)
            cfp.release()
            bzrr, bzri, bzir, bzii, bbrp, bbip = bcs

            # ---------------- phase B: apply from resident bf16 ----------
            for r in range(T):
                rows = slice(r * P, (r + 1) * P)
                R, I = res_r[r], res_i[r]
                t1 = inp.tile([P, D], bf16, tag="i1", name=f"t1_{r}")
                t2 = inp.tile([P, D], bf16, tag="i2", name=f"t2_{r}")
                # yr = zrr*xr + zri*xi + br'   yi = zir*xr + zii*xi + bi'
                nc.vector.tensor_tensor(t1[:, 0:D], R[:], bzrr[:], Alu.mult)
                nc.gpsimd.tensor_tensor(t2[:, 0:D], I[:], bzri[:], Alu.mult)
                nc.vector.tensor_tensor(R[:], R[:], bzir[:], Alu.mult)
                nc.vector.tensor_tensor(I[:], I[:], bzii[:], Alu.mult)
                nc.vector.tensor_tensor(t1[:, 0:D], t1[:, 0:D], t2[:, 0:D],
                                        Alu.add)
                nc.gpsimd.tensor_tensor(t2[:, 0:D], t1[:, 0:D], bbrp[:],
                                        Alu.add)
                nc.vector.tensor_tensor(R[:], R[:], I[:], Alu.add)
                nc.vector.tensor_tensor(R[:], R[:], bbip[:], Alu.add)
                nc.sync.dma_start(out=yr[rows, :], in_=t2[:, 0:D])
                nc.sync.dma_start(out=yi[rows, :], in_=R[:])

    nc.compile()
    return nc


def get_nc():
    if "nc" not in _CACHE:
        _CACHE["nc"] = _build()
    return _CACHE["nc"]


def kernel(xr, xi, Wrr, Wri, Wii, Br, Bi):
    from concourse import bass_utils

    nc = get_nc()
    xr2 = np.ascontiguousarray(np.asarray(xr), dtype=np.float32).reshape(N, D)
    xi2 = np.ascontiguousarray(np.asarray(xi), dtype=np.float32).reshape(N, D)
    params = {
        "wrr": np.ascontiguousarray(np.asarray(Wrr), dtype=np.float32),
        "wri": np.ascontiguousarray(np.asarray(Wri), dtype=np.float32),
        "wii": np.ascontiguousarray(np.asarray(Wii), dtype=np.float32),
        "br": np.ascontiguousarray(np.asarray(Br), dtype=np.float32),
        "bi": np.ascontiguousarray(np.asarray(Bi), dtype=np.float32),
    }
    in_maps = []
    for r in range(N_CORES):
        m = {"xr": xr2[r * NS:(r + 1) * NS], "xi": xi2[r * NS:(r + 1) * NS]}
        m.update(params)
        in_maps.append(m)

    res = bass_utils.run_bass_kernel_spmd(nc, in_maps,
                                          core_ids=list(range(N_CORES)))
    yr_ = np.concatenate(
        [np.asarray(res.results[r]["yr"]).astype(np.float32)
         for r in range(N_CORES)], axis=0)
    yi_ = np.concatenate(
        [np.asarray(res.results[r]["yi"]).astype(np.float32)
         for r in range(N_CORES)], axis=0)
    return yr_.reshape(N, C, F), yi_.reshape(N, C, F)
